# revision 22
# baseline (speedup 1.0000x reference)
"""MoE router (linear gate -> softmax -> top-8 indices) on 8 Trainium2 cores.

Strategy (data-parallel over tokens, W replicated):
  - Each core gets 2048 tokens. x is pre-transposed on the host so each core
    receives x^T [4096, 2048] — the PE needs the contraction dim (d_model) on
    partitions and fp32 has no DMA-transpose path, so transposing on-chip
    would double PE work.
  - Softmax is strictly monotonic, so top-k of softmax(logits) == top-k of
    logits; the softmax is skipped entirely.
  - The gate matmul runs in float32r (fp20: 1+8+11) which streams at 1
    cycle/row vs fp32's 4, using an exactly-compensated split:
        x = x_hi + x_lo,  W = w_hi + w_lo   (each half fp20-representable)
        logits = w_hi·x_hi + w_hi·x_lo + w_lo·x_hi   (fp32 PSUM)
    The dropped w_lo·x_lo term is O(2^-24) relative — fp32-level accuracy
    (validated on HW: max err 1.5e-7 vs fp32's 1.2e-7). W is split on the
    host. x needs NO explicit hi: the PE rounds f32r operands on the fly
    (HW-validated), so the raw x chunk declared f32r IS the x_hi operand;
    x_lo takes one engine op per chunk, x - f32r_view(x), with DVE/GpSimd
    read-rounding the f32r input (HW-validated, bit-identical to host RNE).
    HBM traffic stays 4 bytes/element.
  - PE work is 2 passes per chunk, not 3: the stationary is [w_hi | w_lo]
    [128, 128], so pass A (moving x_hi) yields w_hi·x_hi in PSUM rows 0-63
    AND w_lo·x_hi in rows 64-127 from one moving stream; pass B (moving
    x_lo) uses only the w_hi half into rows 0-63. The two halves are summed
    after the tail transpose, where they sit in the free dim.
  - Streaming: 32 x 1 MiB DMAs (one 128-row contraction chunk each, 358 GB/s
    measured), per-chunk ACT/DVE split, PE accumulates 4 [128, 512] PSUM
    logit tiles across all 32 chunks.
  - Top-8: PE-transpose the logit tiles to [128 tokens, 128], DVE-add the
    two 64-wide halves, then DVE Max8 / MaxIndex produce the 8 largest
    values and indices per token (descending, ties -> lowest index, matching
    jax.lax.top_k). Indices are staged in SBUF and written with one DMA.
"""

import numpy as np

import concourse.bass as bass
import concourse.mybir as mybir
import concourse.tile as tile
from concourse import bacc
from concourse.bass_utils import run_bass_kernel_spmd
from concourse.masks import make_identity

N_CORES = 8
N_TOKENS = 16384
D_MODEL = 4096
N_EXPERTS = 64
TOP_K = 8

TPC = N_TOKENS // N_CORES      # tokens per core (2048)
GROUP = 512                    # tokens per matmul (max 4-byte moving dim)
N_GROUPS = TPC // GROUP        # 4
N_CHUNK = D_MODEL // 128       # 32 contraction chunks
N_BLK = TPC // 128             # 16 x 128-token output blocks

F32 = mybir.dt.float32
F32R = mybir.dt.float32r
U32 = mybir.dt.uint32

_CACHE: dict = {}


def _build_program():
    nc = bacc.Bacc(
        "TRN2", target_bir_lowering=False, debug=False, num_devices=N_CORES
    )
    xt_d = nc.dram_tensor("xt", [D_MODEL, TPC], F32, kind="ExternalInput")
    # [w_hi | w_lo] packed on host: [128, 32, 128] with
    # [p, k, e]      = W_hi[e, k*128+p]  for e < 64
    # [p, k, 64+e]   = W_lo[e, k*128+p]
    ww_d = nc.dram_tensor(
        "ww", [128, N_CHUNK * 2 * N_EXPERTS], F32R, kind="ExternalInput"
    )
    # idx laid out [128 partitions, 16 blocks, 8] — host unpermutes to [2048, 8]
    idx_d = nc.dram_tensor("idx", [128, N_BLK * TOP_K], U32, kind="ExternalOutput")

    with tile.TileContext(nc) as tc:
        with (
            tc.tile_pool(name="const", bufs=1) as const_pool,
            tc.tile_pool(name="xin", bufs=5) as x_pool,
            tc.tile_pool(name="hi", bufs=4) as hi_pool,
            tc.tile_pool(name="lo", bufs=4) as lo_pool,
            tc.tile_pool(name="lg_ps", bufs=1, space="PSUM") as lg_ps_pool,
            tc.tile_pool(name="lt_ps", bufs=2, space="PSUM") as lt_ps_pool,
            tc.tile_pool(name="small", bufs=2 * 4) as small_pool,
        ):
            ident = const_pool.tile([128, 128], F32)
            make_identity(nc, ident[:])
            # W DMA goes on the scalar HWDGE ring so x chunk 0 (sync ring)
            # isn't queued behind it.
            ww_sb = const_pool.tile([128, N_CHUNK, 2 * N_EXPERTS], F32R)
            ww_view = ww_d.ap().rearrange("p (k e) -> p k e", k=N_CHUNK)
            half = N_CHUNK // 2
            nc.scalar.dma_start(ww_sb[:, :half], ww_view[:, :half])
            nc.scalar.dma_start(ww_sb[:, half:], ww_view[:, half:])
            idx_stage = const_pool.tile([128, N_BLK, TOP_K], U32)

            lg_ps = [
                lg_ps_pool.tile(
                    [2 * N_EXPERTS, GROUP], F32, name=f"lg{g}", tag=f"lg{g}"
                )
                for g in range(N_GROUPS)
            ]

            xt_view = xt_d.ap().rearrange("(k p) t -> p k t", p=128)
            for k in range(N_CHUNK):
                x_sb = x_pool.tile([128, TPC], F32)
                nc.sync.dma_start(x_sb[:], xt_view[:, k, :])
                # hi = round_f32r(x) on ACT (engines round by the MEMORY
                # dtype: an f32r tile rounds on every engine read, so the
                # raw x tile must stay f32-declared and hi gets its own
                # f32r tile). lo = x - hi on DVE, write-rounded (exact).
                # Chunk 0 is sliced so the first matmul group starts after
                # ~1/4 of the split latency.
                hi = hi_pool.tile([128, TPC], F32R)
                lo = lo_pool.tile([128, TPC], F32R)
                n_sl = 4 if k == 0 else 1
                for s in range(n_sl):
                    ssl = slice(s * TPC // n_sl, (s + 1) * TPC // n_sl)
                    nc.scalar.copy(hi[:, ssl], x_sb[:, ssl])
                    nc.vector.tensor_tensor(
                        lo[:, ssl], x_sb[:, ssl], hi[:, ssl].bitcast(F32),
                        mybir.AluOpType.subtract,
                    )
                # pass A: [w_hi|w_lo]·hi -> all 128 PSUM rows
                # pass B: w_hi·lo -> rows 0-63 only
                # chunk 0 must OPEN each tile with a full-tile start;
                # chunk 31 must CLOSE each tile with a full-tile stop.
                for g in range(N_GROUPS):
                    sl = slice(g * GROUP, (g + 1) * GROUP)
                    if k == 0:
                        nc.tensor.matmul(
                            lg_ps[g][:], ww_sb[:, k], hi[:, sl],
                            start=True, stop=False,
                        )
                        nc.tensor.matmul(
                            lg_ps[g][: N_EXPERTS], ww_sb[:, k, :N_EXPERTS],
                            lo[:, sl], start=False, stop=False,
                        )
                    elif k < N_CHUNK - 1:
                        nc.tensor.matmul(
                            lg_ps[g][:], ww_sb[:, k], hi[:, sl],
                            start=False, stop=False,
                        )
                        nc.tensor.matmul(
                            lg_ps[g][: N_EXPERTS], ww_sb[:, k, :N_EXPERTS],
                            lo[:, sl], start=False, stop=False,
                        )
                    else:
                        nc.tensor.matmul(
                            lg_ps[g][: N_EXPERTS], ww_sb[:, k, :N_EXPERTS],
                            lo[:, sl], start=False, stop=False,
                        )
                        nc.tensor.matmul(
                            lg_ps[g][:], ww_sb[:, k], hi[:, sl],
                            start=False, stop=True,
                        )

            for g in range(N_GROUPS):
                lg_sb = small_pool.tile([2 * N_EXPERTS, GROUP], F32, tag="lgsb")
                nc.scalar.copy(lg_sb[:], lg_ps[g][:])
                for b in range(GROUP // 128):
                    lt_ps = lt_ps_pool.tile([128, 2 * N_EXPERTS], F32)
                    nc.tensor.transpose(
                        lt_ps[:],
                        lg_sb[:, b * 128 : (b + 1) * 128],
                        ident[:],
                    )
                    lt_h = small_pool.tile([128, N_EXPERTS], F32, tag="lth")
                    nc.scalar.copy(lt_h[:], lt_ps[:, :N_EXPERTS])
                    lt_sb = small_pool.tile([128, N_EXPERTS], F32, tag="ltsb")
                    nc.vector.tensor_tensor(
                        lt_sb[:],
                        lt_h[:],
                        lt_ps[:, N_EXPERTS:],
                        mybir.AluOpType.add,
                    )
                    vals = small_pool.tile([128, TOP_K], F32, tag="vals")
                    nc.vector.max(vals[:], lt_sb[:])
                    nc.vector.max_index(
                        idx_stage[:, g * (GROUP // 128) + b, :], vals[:], lt_sb[:]
                    )

            nc.sync.dma_start(
                idx_d.ap().rearrange("p (b k) -> p b k", b=N_BLK), idx_stage[:]
            )

    nc.compile()
    return nc


def _get_program():
    if "nc" not in _CACHE:
        _CACHE["nc"] = _build_program()
    return _CACHE["nc"]


def _round_f32r(a: np.ndarray) -> np.ndarray:
    """Round fp32 -> fp20 (1+8+11 float32r), RNE, kept as fp32 bit pattern."""
    u = np.ascontiguousarray(a, dtype=np.float32).view(np.uint32)
    low = u & np.uint32(0x00000FFF)
    base = u & np.uint32(0xFFFFF000)
    half = np.uint32(0x800)
    lsb = (u >> np.uint32(12)) & np.uint32(1)
    round_up = (low > half) | ((low == half) & (lsb == 1))
    return (base + np.where(round_up, np.uint32(0x1000), np.uint32(0))).view(
        np.float32
    )


def _pack_ww(W: np.ndarray) -> np.ndarray:
    # [64, 4096] -> [128, 32*128]: [p, k*128+e] = W_hi[e, k*128+p],
    #                              [p, k*128+64+e] = W_lo[e, k*128+p]
    wt = (
        W.astype(np.float32, copy=False)
        .T.reshape(N_CHUNK, 128, N_EXPERTS)
        .transpose(1, 0, 2)
    )  # [128, 32, 64]
    wh = _round_f32r(wt)
    wl = _round_f32r(wt - wh)
    ww = np.concatenate([wh.reshape(128, N_CHUNK, N_EXPERTS),
                         wl.reshape(128, N_CHUNK, N_EXPERTS)], axis=2)
    return np.ascontiguousarray(ww.reshape(128, N_CHUNK * 2 * N_EXPERTS))


def _make_in_maps(x: np.ndarray, W: np.ndarray) -> list:
    x = np.asarray(x, dtype=np.float32)
    ww = _pack_ww(W)
    return [
        {
            "xt": np.ascontiguousarray(x[c * TPC : (c + 1) * TPC].T),
            "ww": ww,
        }
        for c in range(N_CORES)
    ]


def kernel(x: np.ndarray, W: np.ndarray) -> np.ndarray:
    nc = _get_program()
    in_maps = _make_in_maps(x, W)
    res = run_bass_kernel_spmd(nc, in_maps, core_ids=list(range(N_CORES)))
    out = np.concatenate(
        [
            res.results[c]["idx"]
            .reshape(128, N_BLK, TOP_K)
            .transpose(1, 0, 2)
            .reshape(TPC, TOP_K)
            for c in range(N_CORES)
        ],
        axis=0,
    )
    return out.astype(np.int32)
